# revision 13
# baseline (speedup 1.0000x reference)
"""Causal self-attention (B=4, T=2048, C=1024, H=16) on 8 NeuronCores.

Sharding: core c handles batch b = c//2 and head-half half = c%2 (8 heads,
512 channels). QKV projections are column-parallel, output projection is
row-parallel (Megatron); the two per-batch output partials are summed on host.

v2 design (bf16 compute, f32 PSUM accumulation):
  - All matmul operands bf16: enables compiler fast-weight-load (FWL), halves
    DMA traffic and SBUF footprint vs f32r. Verified numerically: metric
    ~4e-3 vs 2e-2 tolerance.
  - Bias matmuls eliminated: bk is softmax-invariant (dropped exactly), bv is
    folded into bp host-side (bp_eff = bp + Wp_slice @ bv), bq is fused into
    the PSUM->SBUF copy as a per-partition tensor_scalar add.
  - Interleaved schedule: v for all heads first, then per m-tile (2 heads):
    q/k projections followed immediately by attention for those heads, so the
    Activation engine (exp, the phase-2 bottleneck) starts ~30us in and runs
    concurrently with remaining projections on the PE.
  - yT stays in SBUF (no DRAM bounce). Odd heads (partitions 64-127 of the
    per-m-tile yT tile) are placed via a SBUF->SBUF DMA since DVE lanes
    cannot cross partitions.
  - Output projection computes outT [C, T] (queries on the free axis) so bp
    is a per-partition add fused into the PSUM->SBUF copy; host transposes.
  - Softmax uses a fixed max of 0 (scores ~N(0,1), exp safe in f32); the
    denominator comes from the ones-column appended to each head's v (vAug),
    so one [65 x N] matmul accumulates numerator and denominator together.
"""

import sys
import types

import numpy as np
from contextlib import ExitStack

import concourse.bass as bass
import concourse.mybir as mybir
import concourse.tile as tile
from concourse import bacc
from concourse.bass_utils import run_bass_kernel_spmd

# If the environment sets BASS_TRACE but ships only the antenv stub (no
# axon_hooks), run_bass_kernel_spmd would crash on import. Provide the
# graceful "no hook registered" fallback only when the real module is absent.
try:  # pragma: no cover
    import antenv.axon_hooks  # noqa: F401
except ImportError:  # pragma: no cover
    import antenv

    _stub = types.ModuleType("antenv.axon_hooks")
    _stub.get_axon_ntff_profile_hook = lambda: None
    sys.modules["antenv.axon_hooks"] = _stub
    antenv.axon_hooks = _stub

F32 = mybir.dt.float32
BF16 = mybir.dt.bfloat16
EXP = mybir.ActivationFunctionType.Exp

B, T, C, H = 4, 2048, 1024, 16
HD = C // H              # 64 head dim
N_CORES = 8
HPC = H // 2             # 8 heads per core
MPC = C // 2             # 512 channels per core
MT = MPC // 128          # 4 m-tiles per core
CT = C // 128            # 8 contraction tiles
TC = T // 512            # 4 t-chunks
TT = T // 128            # 16 t-tiles
SCALE = float(1.0 / np.sqrt(HD))

_CACHE = {}


def _build(bench_loops=None):
    import contextlib

    nc = bacc.Bacc()
    xT = nc.declare_dram_parameter("xT", [C, T], BF16, isOutput=False)
    wqT = nc.declare_dram_parameter("wqT", [C, MPC], BF16, isOutput=False)
    wkT = nc.declare_dram_parameter("wkT", [C, MPC], BF16, isOutput=False)
    wvT = nc.declare_dram_parameter("wvT", [C, MPC], BF16, isOutput=False)
    wpT = nc.declare_dram_parameter("wpT", [MPC, C], BF16, isOutput=False)
    bqd = nc.declare_dram_parameter("bqv", [128, MT], F32, isOutput=False)
    bpd = nc.declare_dram_parameter("bpv", [128, CT], F32, isOutput=False)
    mask01d = nc.declare_dram_parameter("mask01", [128, 128], BF16, isOutput=False)
    outp = nc.declare_dram_parameter("out", [C, T], F32, isOutput=True)

    with tile.TileContext(nc) as tc:
        with ExitStack() as ctx:
            persist = ctx.enter_context(tc.tile_pool(name="persist", bufs=1))
            pool_P = ctx.enter_context(tc.tile_pool(name="pool_P", bufs=3))
            pool_tail = ctx.enter_context(tc.tile_pool(name="pool_tail", bufs=2))
            pool_out = ctx.enter_context(tc.tile_pool(name="pool_out", bufs=3))
            ps_proj = ctx.enter_context(tc.tile_pool(name="ps_proj", bufs=2, space="PSUM"))
            ps_s = ctx.enter_context(tc.tile_pool(name="ps_s", bufs=2, space="PSUM"))
            ps_y = ctx.enter_context(tc.tile_pool(name="ps_y", bufs=1, space="PSUM"))

            # ---- weights / constants (loaded once, outside the bench loop) ----
            # SP queue carries what the first matmuls need (wv); the bulk
            # weight loads go on the Activation engine's HWDGE queue so the
            # two descriptor streams drain in parallel and x (issued on SP
            # inside the loop) isn't stuck behind 6MB of weights.
            wv_t = [persist.tile([128, MPC], BF16, name=f"wv{c}") for c in range(CT)]
            for c in range(CT):
                nc.scalar.dma_start(out=wv_t[c], in_=wvT[c * 128:(c + 1) * 128, :])
            xs = [persist.tile([128, T], BF16, name=f"xs{c}") for c in range(CT)]
            wq_t = [persist.tile([128, MPC], BF16, name=f"wq{c}") for c in range(CT)]
            wk_t = [persist.tile([128, MPC], BF16, name=f"wk{c}") for c in range(CT)]
            for c in range(CT):
                nc.scalar.dma_start(out=wq_t[c], in_=wqT[c * 128:(c + 1) * 128, :])
                nc.scalar.dma_start(out=wk_t[c], in_=wkT[c * 128:(c + 1) * 128, :])
            wp_t = [persist.tile([128, C], BF16, name=f"wp{m}") for m in range(MT)]
            for m in range(MT):
                nc.scalar.dma_start(out=wp_t[m], in_=wpT[m * 128:(m + 1) * 128, :])
            mask01_sb = persist.tile([128, 128], BF16, name="mask01_sb")
            nc.scalar.dma_start(out=mask01_sb, in_=mask01d[:, :])
            bq_sb = persist.tile([128, MT], F32, name="bq_sb")
            nc.scalar.dma_start(out=bq_sb, in_=bqd[:, :])
            bp_sb = persist.tile([128, CT], F32, name="bp_sb")
            nc.scalar.dma_start(out=bp_sb, in_=bpd[:, :])

            # ---- persistent activations ----
            # kT is stored per head, zero-padded to the full 128 contraction
            # rows (even head: data rows 0..63, zeros below; odd head: zeros
            # on top, data rows 64..127 — matching its position in the k
            # projection PSUM so the DVE copy stays partition-aligned). The S
            # matmul then always contracts over K=128 with a full 128-column
            # stationary, which measures substantially faster per instruction
            # than the K=64 quad-tile form.
            qT = [persist.tile([128, T], BF16, name=f"qT{m}") for m in range(MT)]
            kTp = [persist.tile([128, T], BF16, name=f"kTp{h}") for h in range(H // 2)]
            vAug = [persist.tile([128, HPC * (HD + 1)], BF16, name=f"vAug{t}") for t in range(TT)]
            yT = [persist.tile([128, T], BF16, name=f"yT{m}") for m in range(MT)]
            for h in range(H // 2):
                so = (h % 2) * HD
                nc.vector.memset(kTp[h][HD - so:128 - so, :], 0.0)

            # ones column of vAug: set once; the in-loop v copies only touch
            # columns 0..63 of each head slice, so this survives bench loops.
            ones_f32 = persist.tile([128, 8], F32, name="ones_f32")
            nc.vector.memset(ones_f32, 1.0)
            ones_bf = persist.tile([128, 8], BF16, name="ones_bf")
            nc.vector.tensor_copy(ones_bf, ones_f32)
            for t_ in range(TT):
                va = vAug[t_].rearrange("p (h w) -> p h w", w=HD + 1)
                nc.vector.tensor_copy(va[:, :, HD], ones_bf)

            rep = ctx.enter_context(
                tc.For_i(0, bench_loops, 1) if bench_loops else contextlib.nullcontext())

            # x arrives in two halves per c-tile: the first covers everything
            # v and the mt=0 projections need, so compute starts early.
            for half in range(2):
                t0 = half * 1024
                for c in range(CT):
                    nc.sync.dma_start(out=xs[c][:, t0:t0 + 1024],
                                      in_=xT[c * 128:(c + 1) * 128, t0:t0 + 1024])

            # ---- v for all heads (x-stationary): psum[t 128, m 512] ----
            for tt in range(TT):
                psv = ps_proj.tile([128, MPC], F32, name=f"psv_{tt}", tag="proj")
                for c in range(CT):
                    nc.tensor.matmul(psv, xs[c][:, tt * 128:(tt + 1) * 128], wv_t[c],
                                     start=(c == 0), stop=(c == CT - 1))
                va = vAug[tt].rearrange("p (h w) -> p h w", w=HD + 1)
                nc.vector.tensor_copy(va[:, :, 0:HD],
                                      psv.rearrange("p (h w) -> p h w", w=HD))

            # ---- emission helpers ----
            def _proj_gen(mt):
                """q/k projection for m-tile mt as a stream of PE/DVE thunks,
                so it can be interleaved into the previous m-tile's attention
                (the attention j-loop is Act-paced; the PE has slack)."""
                for wt, bias in ((wq_t, True), (wk_t, False)):
                    for tch in range(TC):
                        t0 = tch * 512
                        ps = ps_proj.tile([128, 512], F32, name=f"ps_{mt}_{bias}_{tch}",
                                          tag="proj")
                        for c in range(CT):
                            yield lambda ps=ps, c=c, wt=wt, t0=t0: nc.tensor.matmul(
                                ps, wt[c][:, mt * 128:(mt + 1) * 128], xs[c][:, t0:t0 + 512],
                                start=(c == 0), stop=(c == CT - 1))
                        if bias:
                            yield lambda ps=ps, t0=t0: nc.vector.tensor_scalar_add(
                                qT[mt][:, t0:t0 + 512], ps, bq_sb[:, mt:mt + 1])
                        else:
                            # split per head into the zero-padded kT tiles;
                            # both copies stay on their own partitions.
                            yield lambda ps=ps, t0=t0: nc.vector.tensor_copy(
                                kTp[2 * mt][0:HD, t0:t0 + 512], ps[0:HD, :])
                            yield lambda ps=ps, t0=t0: nc.vector.tensor_copy(
                                kTp[2 * mt + 1][HD:128, t0:t0 + 512], ps[HD:128, :])

            def _phase3_gen(tp):
                """Output-projection t-pair tp as a thunk stream (filler for
                the last m-tile's attention once its inputs are ready)."""
                for n in range(CT):
                    o_sb = pool_out.tile([128, 1024], F32, name=f"o_{tp}_{n}", tag="o")
                    for half in range(2):
                        t0 = tp * 1024 + half * 512
                        pso = ps_proj.tile([128, 512], F32, name=f"pso_{tp}_{n}_{half}",
                                           tag="proj")
                        for m in range(MT):
                            yield lambda pso=pso, m=m, n=n, t0=t0: nc.tensor.matmul(
                                pso, wp_t[m][:, n * 128:(n + 1) * 128], yT[m][:, t0:t0 + 512],
                                start=(m == 0), stop=(m == MT - 1))
                        yield lambda o_sb=o_sb, pso=pso, n=n, half=half: \
                            nc.vector.tensor_scalar_add(
                                o_sb[:, half * 512:(half + 1) * 512], pso, bp_sb[:, n:n + 1])
                    eng = nc.sync if n % 2 == 0 else nc.scalar
                    yield lambda eng=eng, o_sb=o_sb, n=n, tp=tp: eng.dma_start(
                        out=outp[n * 128:(n + 1) * 128, tp * 1024:(tp + 1) * 1024], in_=o_sb)

            def _drain(filler):
                for f in filler:
                    f()

            def _attention(h, ic, filler):
                mt, so = h // 2, (h % 2) * HD
                kT_h, qT_h = kTp[h], qT[mt]
                i0 = ic * 1024
                jmax = 8 * ic + 7
                psy = ps_y.tile([HD + 1, 1024], F32, name=f"psy_{h}_{ic}", tag="psy")
                # software-pipelined emission: y(j-1) lands after S(j)/exp(j),
                # with ~2 filler PE ops per j to absorb the Act-engine pacing.
                pend = None

                def _emit_y(j, P):
                    lo = max(0, j * 128 - i0)
                    for a, b in ((lo, 512), (max(lo, 512), 1024)):
                        if a < b:
                            nc.tensor.matmul(psy[:, a:b],
                                             vAug[j][:, h * (HD + 1):(h + 1) * (HD + 1)],
                                             P[:, a:b],
                                             start=(j == 0), stop=(j == jmax))

                for j in range(jmax + 1):
                    lo = max(0, j * 128 - i0)
                    pss = ps_s.tile([128, 1024], F32, name=f"pss_{h}_{ic}_{j}", tag="S")
                    for a, b in ((lo, 512), (max(lo, 512), 1024)):
                        if a < b:
                            nc.tensor.matmul(pss[:, a:b],
                                             kT_h[:, j * 128:(j + 1) * 128],
                                             qT_h[:, i0 + a:i0 + b],
                                             start=True, stop=True)
                    P = pool_P.tile([128, 1024], BF16, name=f"P_{h}_{ic}_{j}", tag="P")
                    nc.scalar.activation(out=P[:, lo:1024], in_=pss[:, lo:1024],
                                         func=EXP, scale=SCALE)
                    if j * 128 >= i0:  # diagonal block: in-tile causal mask
                        nc.vector.tensor_mul(P[:, lo:lo + 128], P[:, lo:lo + 128],
                                             mask01_sb)
                    for f in (next(filler, None), next(filler, None)):
                        if f is not None:
                            f()
                    if pend is not None:
                        _emit_y(*pend)
                    pend = (j, P)
                _emit_y(*pend)
                # tail: one copy frees the psum accumulator; normalize runs off
                # the critical path on DVE/Pool (kept off the Act engine, which
                # paces the exp pipeline).
                yu = pool_tail.tile([HD + 1, 1024], F32, name=f"yu_{h}_{ic}", tag="yu")
                nc.vector.tensor_copy(yu, psy)
                r32 = pool_tail.tile([1, 1024], F32, name=f"r32_{h}_{ic}", tag="r32")
                nc.vector.reciprocal(r32, yu[HD:HD + 1, :])
                rb = pool_tail.tile([HD, 1024], F32, name=f"rb_{h}_{ic}", tag="rb")
                nc.gpsimd.partition_broadcast(rb, r32)
                if so == 0:
                    nc.vector.tensor_mul(yT[mt][0:HD, i0:i0 + 1024], yu[0:HD, :], rb)
                else:
                    # DVE lanes cannot cross partitions; stage and DMA into
                    # partitions 64..127 of yT[mt].
                    yst = pool_tail.tile([HD, 1024], BF16, name=f"yst_{h}_{ic}", tag="yst")
                    nc.vector.tensor_mul(yst, yu[0:HD, :], rb)
                    nc.sync.dma_start(out=yT[mt][so:so + HD, i0:i0 + 1024], in_=yst)

            # ---- schedule ----
            # mt=0 projections run up front (nothing to interleave them with);
            # projections for mt+1 ride inside mt's attention; the last m-tile
            # runs both heads' ic=1 passes first so the output projection's
            # t-pair 1 can fill its ic=0 passes, and t-pair 0 drains at the end.
            _drain(_proj_gen(0))
            empty = iter(())
            for mt in range(MT):
                filler = _proj_gen(mt + 1) if mt + 1 < MT else empty
                if mt < MT - 1:
                    for hh in range(2):
                        for ic in (0, 1):
                            _attention(2 * mt + hh, ic, filler)
                    _drain(filler)
                else:
                    _attention(2 * mt, 1, empty)
                    _attention(2 * mt + 1, 1, empty)
                    p3 = _phase3_gen(1)
                    _attention(2 * mt, 0, p3)
                    _attention(2 * mt + 1, 0, p3)
                    _drain(p3)
                    _drain(_phase3_gen(0))
    nc.finalize()
    return nc


def _get_nc(bench_loops=None, phases=None):
    key = ("nc", bench_loops)
    if key not in _CACHE:
        _CACHE[key] = _build(bench_loops)
    return _CACHE[key]


def make_in_maps(x, Wk, bk, Wq, bq, Wv, bv, Wp, bp):
    import ml_dtypes
    bf16 = ml_dtypes.bfloat16

    x = np.asarray(x, dtype=np.float32)
    Wk, Wq, Wv, Wp = (np.asarray(a, dtype=np.float32) for a in (Wk, Wq, Wv, Wp))
    bk, bq, bv, bp = (np.asarray(a, dtype=np.float32) for a in (bk, bq, bv, bp))

    mask01 = np.where(np.tril(np.ones((128, 128), dtype=bool)).T, 1.0, 0.0).astype(bf16)
    xT_b = [np.ascontiguousarray(x[b].T.astype(bf16)) for b in range(B)]
    in_maps = []
    for c in range(N_CORES):
        b, half = c // 2, c % 2
        hs = half * MPC
        # bk drops out of softmax exactly (adds a per-query constant to every
        # score). bv folds into the output bias: (y + bv) @ WpT = y @ WpT +
        # Wp_slice @ bv, applied once per core on its own column slice.
        bp_eff = (bp if half == 0 else np.zeros_like(bp)) + Wp[:, hs:hs + MPC] @ bv[hs:hs + MPC]
        in_maps.append({
            "xT": xT_b[b],
            "wqT": np.ascontiguousarray(Wq[hs:hs + MPC, :].T.astype(bf16)),
            "wkT": np.ascontiguousarray(Wk[hs:hs + MPC, :].T.astype(bf16)),
            "wvT": np.ascontiguousarray(Wv[hs:hs + MPC, :].T.astype(bf16)),
            "wpT": np.ascontiguousarray(Wp[:, hs:hs + MPC].T.astype(bf16)),
            "bqv": np.ascontiguousarray(bq[hs:hs + MPC].reshape(MT, 128).T.astype(np.float32)),
            "bpv": np.ascontiguousarray(bp_eff.reshape(CT, 128).T.astype(np.float32)),
            "mask01": mask01,
        })
    return in_maps


def kernel(x, Wk, bk, Wq, bq, Wv, bv, Wp, bp, **run_kwargs):
    in_maps = make_in_maps(x, Wk, bk, Wq, bq, Wv, bv, Wp, bp)
    nc = _get_nc()
    res = run_bass_kernel_spmd(nc, in_maps, core_ids=list(range(N_CORES)), **run_kwargs)
    out = np.empty((B, T, C), dtype=np.float32)
    for b in range(B):
        out[b] = (res.results[2 * b]["out"] + res.results[2 * b + 1]["out"]).T
    if run_kwargs:
        kernel.last_results = res
    return out


# revision 15
# speedup vs baseline: 1.1777x; 1.1777x over previous
"""Causal self-attention (B=4, T=2048, C=1024, H=16) on 8 NeuronCores.

Sharding: core c handles batch b = c//2 and head-half half = c%2 (8 heads,
512 channels). QKV projections are column-parallel, output projection is
row-parallel (Megatron); the two per-batch output partials are summed on host.

v2 design (bf16 compute, f32 PSUM accumulation):
  - All matmul operands bf16: enables compiler fast-weight-load (FWL), halves
    DMA traffic and SBUF footprint vs f32r. Verified numerically: metric
    ~4e-3 vs 2e-2 tolerance.
  - Bias matmuls eliminated: bk is softmax-invariant (dropped exactly), bv is
    folded into bp host-side (bp_eff = bp + Wp_slice @ bv), bq is fused into
    the PSUM->SBUF copy as a per-partition tensor_scalar add.
  - Interleaved schedule: v for all heads first, then per m-tile (2 heads):
    q/k projections followed immediately by attention for those heads, so the
    Activation engine (exp, the phase-2 bottleneck) starts ~30us in and runs
    concurrently with remaining projections on the PE.
  - yT stays in SBUF (no DRAM bounce). Odd heads (partitions 64-127 of the
    per-m-tile yT tile) are placed via a SBUF->SBUF DMA since DVE lanes
    cannot cross partitions.
  - Output projection computes outT [C, T] (queries on the free axis) so bp
    is a per-partition add fused into the PSUM->SBUF copy; host transposes.
  - Softmax uses a fixed max of 0 (scores ~N(0,1), exp safe in f32); the
    denominator comes from the ones-column appended to each head's v (vAug),
    so one [65 x N] matmul accumulates numerator and denominator together.
"""

import sys
import types

import numpy as np
from contextlib import ExitStack

import concourse.bass as bass
import concourse.mybir as mybir
import concourse.tile as tile
from concourse import bacc
from concourse.bass_utils import run_bass_kernel_spmd

# If the environment sets BASS_TRACE but ships only the antenv stub (no
# axon_hooks), run_bass_kernel_spmd would crash on import. Provide the
# graceful "no hook registered" fallback only when the real module is absent.
try:  # pragma: no cover
    import antenv.axon_hooks  # noqa: F401
except ImportError:  # pragma: no cover
    import antenv

    _stub = types.ModuleType("antenv.axon_hooks")
    _stub.get_axon_ntff_profile_hook = lambda: None
    sys.modules["antenv.axon_hooks"] = _stub
    antenv.axon_hooks = _stub

F32 = mybir.dt.float32
BF16 = mybir.dt.bfloat16
EXP = mybir.ActivationFunctionType.Exp

B, T, C, H = 4, 2048, 1024, 16
HD = C // H              # 64 head dim
N_CORES = 8
HPC = H // 2             # 8 heads per core
MPC = C // 2             # 512 channels per core
MT = MPC // 128          # 4 m-tiles per core
CT = C // 128            # 8 contraction tiles
TC = T // 512            # 4 t-chunks
TT = T // 128            # 16 t-tiles
SCALE = float(1.0 / np.sqrt(HD))

_CACHE = {}


def _build(bench_loops=None):
    import contextlib

    nc = bacc.Bacc()
    xT = nc.declare_dram_parameter("xT", [C, T], BF16, isOutput=False)
    wqT = nc.declare_dram_parameter("wqT", [C, MPC], BF16, isOutput=False)
    wkT = nc.declare_dram_parameter("wkT", [C, MPC], BF16, isOutput=False)
    wvT = nc.declare_dram_parameter("wvT", [C, MPC], BF16, isOutput=False)
    wpT = nc.declare_dram_parameter("wpT", [MPC, C], BF16, isOutput=False)
    bqd = nc.declare_dram_parameter("bqv", [128, MT], F32, isOutput=False)
    bpd = nc.declare_dram_parameter("bpv", [128, CT], F32, isOutput=False)
    mask01d = nc.declare_dram_parameter("mask01", [128, 128], BF16, isOutput=False)
    outp = nc.declare_dram_parameter("out", [C, T], F32, isOutput=True)

    with tile.TileContext(nc) as tc:
        with ExitStack() as ctx:
            persist = ctx.enter_context(tc.tile_pool(name="persist", bufs=1))
            pool_P = ctx.enter_context(tc.tile_pool(name="pool_P", bufs=4))
            pool_tail = ctx.enter_context(tc.tile_pool(name="pool_tail", bufs=2))
            pool_out = ctx.enter_context(tc.tile_pool(name="pool_out", bufs=3))
            ps_proj = ctx.enter_context(tc.tile_pool(name="ps_proj", bufs=2, space="PSUM"))
            ps_s = ctx.enter_context(tc.tile_pool(name="ps_s", bufs=2, space="PSUM"))
            ps_y = ctx.enter_context(tc.tile_pool(name="ps_y", bufs=1, space="PSUM"))

            # ---- weights / constants (loaded once, outside the bench loop) ----
            # SP queue carries what the first matmuls need (wv); the bulk
            # weight loads go on the Activation engine's HWDGE queue so the
            # two descriptor streams drain in parallel and x (issued on SP
            # inside the loop) isn't stuck behind 6MB of weights.
            wv_t = [persist.tile([128, MPC], BF16, name=f"wv{c}") for c in range(CT)]
            for c in range(CT):
                nc.scalar.dma_start(out=wv_t[c], in_=wvT[c * 128:(c + 1) * 128, :])
            xs = [persist.tile([128, T], BF16, name=f"xs{c}") for c in range(CT)]
            wq_t = [persist.tile([128, MPC], BF16, name=f"wq{c}") for c in range(CT)]
            wk_t = [persist.tile([128, MPC], BF16, name=f"wk{c}") for c in range(CT)]
            for c in range(CT):
                nc.scalar.dma_start(out=wq_t[c], in_=wqT[c * 128:(c + 1) * 128, :])
                nc.scalar.dma_start(out=wk_t[c], in_=wkT[c * 128:(c + 1) * 128, :])
            wp_t = [persist.tile([128, C], BF16, name=f"wp{m}") for m in range(MT)]
            for m in range(MT):
                nc.scalar.dma_start(out=wp_t[m], in_=wpT[m * 128:(m + 1) * 128, :])
            mask01_sb = persist.tile([128, 128], BF16, name="mask01_sb")
            nc.scalar.dma_start(out=mask01_sb, in_=mask01d[:, :])
            bq_sb = persist.tile([128, MT], F32, name="bq_sb")
            nc.scalar.dma_start(out=bq_sb, in_=bqd[:, :])
            bp_sb = persist.tile([128, CT], F32, name="bp_sb")
            nc.scalar.dma_start(out=bp_sb, in_=bpd[:, :])

            # ---- persistent activations ----
            # kT is stored per head, zero-padded to the full 128 contraction
            # rows (even head: data rows 0..63, zeros below; odd head: zeros
            # on top, data rows 64..127 — matching its position in the k
            # projection PSUM so the DVE copy stays partition-aligned). The S
            # matmul then always contracts over K=128 with a full 128-column
            # stationary, which measures substantially faster per instruction
            # than the K=64 quad-tile form.
            qT = [persist.tile([128, T], BF16, name=f"qT{m}") for m in range(MT)]
            kTp = [persist.tile([128, T], BF16, name=f"kTp{h}") for h in range(H // 2)]
            vAug = [persist.tile([128, HPC * (HD + 1)], BF16, name=f"vAug{t}") for t in range(TT)]
            yT = [persist.tile([128, T], BF16, name=f"yT{m}") for m in range(MT)]
            for h in range(H // 2):
                so = (h % 2) * HD
                nc.vector.memset(kTp[h][HD - so:128 - so, :], 0.0)

            # ones column of vAug: set once; the in-loop v copies only touch
            # columns 0..63 of each head slice, so this survives bench loops.
            ones_f32 = persist.tile([128, 8], F32, name="ones_f32")
            nc.vector.memset(ones_f32, 1.0)
            ones_bf = persist.tile([128, 8], BF16, name="ones_bf")
            nc.vector.tensor_copy(ones_bf, ones_f32)
            for t_ in range(TT):
                va = vAug[t_].rearrange("p (h w) -> p h w", w=HD + 1)
                nc.vector.tensor_copy(va[:, :, HD], ones_bf)

            rep = ctx.enter_context(
                tc.For_i(0, bench_loops, 1) if bench_loops else contextlib.nullcontext())

            # x arrives in two halves per c-tile: the first covers everything
            # v and the mt=0 projections need, so compute starts early.
            for half in range(2):
                t0 = half * 1024
                for c in range(CT):
                    nc.sync.dma_start(out=xs[c][:, t0:t0 + 1024],
                                      in_=xT[c * 128:(c + 1) * 128, t0:t0 + 1024])

            # ---- v for all heads (x-stationary): psum[t 128, m 512] ----
            for tt in range(TT):
                psv = ps_proj.tile([128, MPC], F32, name=f"psv_{tt}", tag="proj")
                for c in range(CT):
                    nc.tensor.matmul(psv, xs[c][:, tt * 128:(tt + 1) * 128], wv_t[c],
                                     start=(c == 0), stop=(c == CT - 1))
                va = vAug[tt].rearrange("p (h w) -> p h w", w=HD + 1)
                nc.vector.tensor_copy(va[:, :, 0:HD],
                                      psv.rearrange("p (h w) -> p h w", w=HD))

            # ---- emission helpers ----
            def _proj_gen(mt):
                """q/k projection for m-tile mt as a stream of PE/DVE thunks,
                so it can be interleaved into the previous m-tile's attention
                (the attention j-loop is Act-paced; the PE has slack)."""
                for wt, bias in ((wq_t, True), (wk_t, False)):
                    for tch in range(TC):
                        t0 = tch * 512
                        ps = ps_proj.tile([128, 512], F32, name=f"ps_{mt}_{bias}_{tch}",
                                          tag="proj")
                        for c in range(CT):
                            yield lambda ps=ps, c=c, wt=wt, t0=t0: nc.tensor.matmul(
                                ps, wt[c][:, mt * 128:(mt + 1) * 128], xs[c][:, t0:t0 + 512],
                                start=(c == 0), stop=(c == CT - 1))
                        if bias:
                            yield lambda ps=ps, t0=t0: nc.vector.tensor_scalar_add(
                                qT[mt][:, t0:t0 + 512], ps, bq_sb[:, mt:mt + 1])
                        else:
                            # split per head into the zero-padded kT tiles;
                            # both copies stay on their own partitions.
                            yield lambda ps=ps, t0=t0: nc.vector.tensor_copy(
                                kTp[2 * mt][0:HD, t0:t0 + 512], ps[0:HD, :])
                            yield lambda ps=ps, t0=t0: nc.vector.tensor_copy(
                                kTp[2 * mt + 1][HD:128, t0:t0 + 512], ps[HD:128, :])

            def _phase3_gen(tp):
                """Output-projection t-pair tp as a thunk stream (filler for
                the last m-tile's attention once its inputs are ready)."""
                for n in range(CT):
                    o_sb = pool_out.tile([128, 1024], F32, name=f"o_{tp}_{n}", tag="o")
                    for half in range(2):
                        t0 = tp * 1024 + half * 512
                        pso = ps_proj.tile([128, 512], F32, name=f"pso_{tp}_{n}_{half}",
                                           tag="proj")
                        for m in range(MT):
                            yield lambda pso=pso, m=m, n=n, t0=t0: nc.tensor.matmul(
                                pso, wp_t[m][:, n * 128:(n + 1) * 128], yT[m][:, t0:t0 + 512],
                                start=(m == 0), stop=(m == MT - 1))
                        yield lambda o_sb=o_sb, pso=pso, n=n, half=half: \
                            nc.vector.tensor_scalar_add(
                                o_sb[:, half * 512:(half + 1) * 512], pso, bp_sb[:, n:n + 1])
                    eng = nc.sync if n % 2 == 0 else nc.scalar
                    yield lambda eng=eng, o_sb=o_sb, n=n, tp=tp: eng.dma_start(
                        out=outp[n * 128:(n + 1) * 128, tp * 1024:(tp + 1) * 1024], in_=o_sb)

            def _drain(filler):
                for f in filler:
                    f()

            def _attention(h, ic, filler):
                mt, so = h // 2, (h % 2) * HD
                kT_h, qT_h = kTp[h], qT[mt]
                i0 = ic * 1024
                jmax = 8 * ic + 7
                psy = ps_y.tile([HD + 1, 1024], F32, name=f"psy_{h}_{ic}", tag="psy")
                # software-pipelined emission, depth 2: y(j-2) lands after
                # S(j)/exp(j), so every cross-engine semaphore (exp -> y,
                # mask -> y) is posted well before the PE consumer arrives.
                # Measured 2x faster per j than depth 1 on hardware.
                pend = []

                def _emit_y(j, P):
                    lo = max(0, j * 128 - i0)
                    for a, b in ((lo, 512), (max(lo, 512), 1024)):
                        if a < b:
                            nc.tensor.matmul(psy[:, a:b],
                                             vAug[j][:, h * (HD + 1):(h + 1) * (HD + 1)],
                                             P[:, a:b],
                                             start=(j == 0), stop=(j == jmax))

                for j in range(jmax + 1):
                    lo = max(0, j * 128 - i0)
                    pss = ps_s.tile([128, 1024], F32, name=f"pss_{h}_{ic}_{j}", tag="S")
                    for a, b in ((lo, 512), (max(lo, 512), 1024)):
                        if a < b:
                            nc.tensor.matmul(pss[:, a:b],
                                             kT_h[:, j * 128:(j + 1) * 128],
                                             qT_h[:, i0 + a:i0 + b],
                                             start=True, stop=True)
                    P = pool_P.tile([128, 1024], BF16, name=f"P_{h}_{ic}_{j}", tag="P")
                    nc.scalar.activation(out=P[:, lo:1024], in_=pss[:, lo:1024],
                                         func=EXP, scale=SCALE)
                    if j * 128 >= i0:  # diagonal block: in-tile causal mask
                        nc.vector.tensor_mul(P[:, lo:lo + 128], P[:, lo:lo + 128],
                                             mask01_sb)
                    for f in (next(filler, None), next(filler, None)):
                        if f is not None:
                            f()
                    pend.append((j, P))
                    if len(pend) > 2:
                        _emit_y(*pend.pop(0))
                for pj in pend:
                    _emit_y(*pj)
                # tail: one copy frees the psum accumulator; normalize runs off
                # the critical path on DVE/Pool (kept off the Act engine, which
                # paces the exp pipeline).
                yu = pool_tail.tile([HD + 1, 1024], F32, name=f"yu_{h}_{ic}", tag="yu")
                nc.vector.tensor_copy(yu, psy)
                r32 = pool_tail.tile([1, 1024], F32, name=f"r32_{h}_{ic}", tag="r32")
                nc.vector.reciprocal(r32, yu[HD:HD + 1, :])
                rb = pool_tail.tile([HD, 1024], F32, name=f"rb_{h}_{ic}", tag="rb")
                nc.gpsimd.partition_broadcast(rb, r32)
                if so == 0:
                    nc.vector.tensor_mul(yT[mt][0:HD, i0:i0 + 1024], yu[0:HD, :], rb)
                else:
                    # DVE lanes cannot cross partitions; stage and DMA into
                    # partitions 64..127 of yT[mt].
                    yst = pool_tail.tile([HD, 1024], BF16, name=f"yst_{h}_{ic}", tag="yst")
                    nc.vector.tensor_mul(yst, yu[0:HD, :], rb)
                    nc.sync.dma_start(out=yT[mt][so:so + HD, i0:i0 + 1024], in_=yst)

            # ---- schedule ----
            # mt=0 projections run up front (nothing to interleave them with);
            # projections for mt+1 ride inside mt's attention; the last m-tile
            # runs both heads' ic=1 passes first so the output projection's
            # t-pair 1 can fill its ic=0 passes, and t-pair 0 drains at the end.
            _drain(_proj_gen(0))
            empty = iter(())
            for mt in range(MT):
                filler = _proj_gen(mt + 1) if mt + 1 < MT else empty
                if mt < MT - 1:
                    for hh in range(2):
                        for ic in (0, 1):
                            _attention(2 * mt + hh, ic, filler)
                    _drain(filler)
                else:
                    _attention(2 * mt, 1, empty)
                    _attention(2 * mt + 1, 1, empty)
                    p3 = _phase3_gen(1)
                    _attention(2 * mt, 0, p3)
                    _attention(2 * mt + 1, 0, p3)
                    _drain(p3)
                    _drain(_phase3_gen(0))
    nc.finalize()
    return nc


def _get_nc(bench_loops=None, phases=None):
    key = ("nc", bench_loops)
    if key not in _CACHE:
        _CACHE[key] = _build(bench_loops)
    return _CACHE[key]


def make_in_maps(x, Wk, bk, Wq, bq, Wv, bv, Wp, bp):
    import ml_dtypes
    bf16 = ml_dtypes.bfloat16

    x = np.asarray(x, dtype=np.float32)
    Wk, Wq, Wv, Wp = (np.asarray(a, dtype=np.float32) for a in (Wk, Wq, Wv, Wp))
    bk, bq, bv, bp = (np.asarray(a, dtype=np.float32) for a in (bk, bq, bv, bp))

    mask01 = np.where(np.tril(np.ones((128, 128), dtype=bool)).T, 1.0, 0.0).astype(bf16)
    xT_b = [np.ascontiguousarray(x[b].T.astype(bf16)) for b in range(B)]
    in_maps = []
    for c in range(N_CORES):
        b, half = c // 2, c % 2
        hs = half * MPC
        # bk drops out of softmax exactly (adds a per-query constant to every
        # score). bv folds into the output bias: (y + bv) @ WpT = y @ WpT +
        # Wp_slice @ bv, applied once per core on its own column slice.
        bp_eff = (bp if half == 0 else np.zeros_like(bp)) + Wp[:, hs:hs + MPC] @ bv[hs:hs + MPC]
        in_maps.append({
            "xT": xT_b[b],
            "wqT": np.ascontiguousarray(Wq[hs:hs + MPC, :].T.astype(bf16)),
            "wkT": np.ascontiguousarray(Wk[hs:hs + MPC, :].T.astype(bf16)),
            "wvT": np.ascontiguousarray(Wv[hs:hs + MPC, :].T.astype(bf16)),
            "wpT": np.ascontiguousarray(Wp[:, hs:hs + MPC].T.astype(bf16)),
            "bqv": np.ascontiguousarray(bq[hs:hs + MPC].reshape(MT, 128).T.astype(np.float32)),
            "bpv": np.ascontiguousarray(bp_eff.reshape(CT, 128).T.astype(np.float32)),
            "mask01": mask01,
        })
    return in_maps


def kernel(x, Wk, bk, Wq, bq, Wv, bv, Wp, bp, **run_kwargs):
    in_maps = make_in_maps(x, Wk, bk, Wq, bq, Wv, bv, Wp, bp)
    nc = _get_nc()
    res = run_bass_kernel_spmd(nc, in_maps, core_ids=list(range(N_CORES)), **run_kwargs)
    out = np.empty((B, T, C), dtype=np.float32)
    for b in range(B):
        out[b] = (res.results[2 * b]["out"] + res.results[2 * b + 1]["out"]).T
    if run_kwargs:
        kernel.last_results = res
    return out
